# revision 38
# baseline (speedup 1.0000x reference)
"""MoE routing kernel for Trainium2 (8 NeuronCores, expert-parallel).

Problem: y[n] = x[n] @ W[index[n]].T + b[index[n]]
  x [16384, 1024] f32, index [16384] i32, W [8, 512, 1024] f32, b [8, 512] f32

Strategy (expert-parallel, dispatch on index during sharding):
  Core e owns expert e. The host groups rows by expert (the all-to-all
  dispatch), packs each core's rows into PE-friendly transposed tiles, and
  each core runs a dense [R,1024] @ [1024,512] matmul with its expert's
  weights. Results are scattered back to original row order on the host.

Device layout per core (one NEFF, SPMD on cores 0-7):
  xT  [RT, 128, 8, 128]  (row-tile, k%128, k-tile, r) — lhsT blocks; a
                         partition line (fixed k) is contiguous in DRAM
  wT  [8, 128, 512]      (k-tile, k, o)               — rhs blocks (moving)
  y   [RT, 128, 512]     (row-tile, r, o)
  For each row-tile: accumulate 8 matmuls over k-tiles into one PSUM bank,
  copy PSUM->SBUF on DVE, DMA out.
"""

from contextlib import ExitStack

import numpy as np

import concourse.bass as bass
import concourse.mybir as mybir
import concourse.tile as tile
from concourse import bacc
from concourse.bass_utils import run_bass_kernel_spmd

N_CORES = 8
D_IN = 1024
D_OUT = 512
KT = D_IN // 128  # 8 k-tiles

# matmul input dtypes (lhsT = x blocks, rhs = W blocks). float16 runs the
# PE at 1 column/cycle with fast weight load (fp32 is 4x slower, fp32r has
# no fast weight load) and halves the input DMA. Accuracy vs the fp32
# reference is ~3e-4 relative (10-bit mantissa; values here are well within
# fp16 range: |x| < ~6, |W| < ~0.06, accumulation in fp32 PSUM).
X_DT = mybir.dt.float16
W_DT = mybir.dt.float16

# Number of PE-warmup dummy matmuls (0 disables). They run in the dead
# window between the engine-body start (~15us) and the first real matmul
# (~17.6us, gated by DMA completions), accumulating HAM busy time so the
# real matmuls run at 2.4 GHz sooner.
WARMUP_MMS = 8

# Skip the construction-time all-engine barrier (earlier first DMA).
SKIP_INIT_BARRIER = False


class _NoInitBarrierBacc(bacc.Bacc):
    """Bacc whose construction-time all-engine barrier is skipped.

    Bass.__init__ ends with an all-engine barrier whose only job is to order
    the const-pool memsets (which this kernel never reads) before the body.
    Skipping it lets each engine enter the body as soon as the runtime
    releases it, so the first DMAs issue ~4us earlier. All body dependencies
    are still fully managed by Tile's semaphores (initialized by the NEFF
    loader, not by engine code).
    """

    def all_engine_barrier(self, *, sem_only: bool = False):
        if not getattr(self, "_init_barrier_skipped", False):
            self._init_barrier_skipped = True
            return None
        return super().all_engine_barrier(sem_only=sem_only)


def build_nc(rt: int, x_dt=None, w_dt=None):
    """Build + compile the per-core Bass program for `rt` row-tiles."""
    x_dt = x_dt or X_DT
    w_dt = w_dt or W_DT
    nc = (_NoInitBarrierBacc if SKIP_INIT_BARRIER else bacc.Bacc)(
        "TRN2",
        target_bir_lowering=False,
        debug=False,
        enable_asserts=False,
        num_devices=N_CORES,
    )
    f32 = mybir.dt.float32
    xT = nc.dram_tensor("xT", [rt, 128, KT * 128], x_dt, kind="ExternalInput").ap()
    wT = nc.dram_tensor("wT", [KT, 128, D_OUT], w_dt, kind="ExternalInput").ap()
    y = nc.dram_tensor("y", [rt, 128, D_OUT], f32, kind="ExternalOutput").ap()

    with tile.TileContext(nc) as tc, ExitStack() as ctx:
        w_pool = ctx.enter_context(tc.tile_pool(name="w", bufs=1))
        x_pool = ctx.enter_context(tc.tile_pool(name="x", bufs=8))
        o_pool = ctx.enter_context(tc.tile_pool(name="o", bufs=8))
        p_pool = ctx.enter_context(tc.tile_pool(name="p", bufs=6, space="PSUM"))

        w_tiles = []
        for kt in range(KT):
            w_tiles.append(
                w_pool.tile([128, D_OUT], w_dt, tag=f"w{kt}", name=f"w{kt}")
            )

        # PE warmup: the HAM clock gate keeps the PE at 1.2 GHz until it has
        # been busy ~3.4us, and re-throttles after ~3.4us idle.
        if WARMUP_MMS:
            warm_pool = ctx.enter_context(tc.tile_pool(name="warm", bufs=1))
            warm_sb = warm_pool.tile(
                [128, D_OUT], x_dt, tag="warm", name="warm_sb"
            )
            nc.vector.memset(warm_sb[:], 0.0)
            warm_ps = p_pool.tile(
                [128, D_OUT], f32, tag="warm_ps", name="warm_ps", bufs=1
            )
            for i in range(WARMUP_MMS):
                nc.tensor.matmul(
                    warm_ps[:], warm_sb[:, :128], warm_sb[:], start=True, stop=True
                )

        # Head: the first H row-tiles are processed k-major (for each
        # k-tile, H matmuls across the row-tiles). A single row-tile
        # consumes one W k-tile per 216ns, but each DMA ring completes a
        # transfer only every ~0.6-1.1us, so a row-major head stalls on W
        # arrivals and the stalls break the HAM busy window (leaving the PE
        # at 1.2 GHz for a core-dependent stretch). With H=4 matmuls per W
        # k-tile the consumption rate (~0.86us/k-tile) stays above the
        # arrival rate, so the head streams gap-free right after the warmup
        # chain. The head x block is loaded k-block-major (one DMA per
        # k-tile pair covering all H row-tiles) to match consumption order.
        # H=6 gives enough slack that even a slow DMA window (HBM
        # contention) cannot stall the stream and re-throttle the PE.
        H = min(6, rt)
        x_tiles = {}
        xh = x_pool.tile(
            [128, (KT // 2) * H * 256], x_dt, tag="xh", name="xh", bufs=1
        )
        for p in range(KT // 2):
            dst = xh[:, p * H * 256 : (p + 1) * H * 256]
            nc.sync.dma_start(
                dst.rearrange("k (j f) -> k j f", j=H),
                xT[0:H, :, p * 256 : (p + 1) * 256].rearrange("j k f -> k j f"),
            )
        for kt in (0, 2, 4, 6):
            nc.scalar.dma_start(w_tiles[kt][:], wT[kt])
        for kt in (1, 3, 5, 7):
            nc.gpsimd.dma_start(w_tiles[kt][:], wT[kt])
        for r in range(H, rt):
            x_t = x_pool.tile([128, KT * 128], x_dt, tag="x", name=f"x{r}")
            nc.sync.dma_start(x_t[:], xT[r])
            x_tiles[r] = x_t

        def store_out(r, psum):
            # Copy + store in halves: the second half's chain (copy ->
            # issue -> transfer -> completion) is what trails the last
            # matmul, and halving it shortens the kernel tail.
            o_t = o_pool.tile([128, D_OUT], f32, tag="o", name=f"o{r}")
            half = D_OUT // 2
            for h in (0, 1):
                sl = slice(h * half, (h + 1) * half)
                nc.vector.tensor_copy(o_t[:, sl], psum[:, sl])
                nc.scalar.dma_start(y[r][:, sl], o_t[:, sl])

        head_psums = [
            p_pool.tile([128, D_OUT], f32, tag="ps", name=f"ps{j}")
            for j in range(H)
        ]
        for kt in range(KT):
            p = kt // 2
            for j in range(H):
                off = p * H * 256 + j * 256 + (kt % 2) * 128
                nc.tensor.matmul(
                    head_psums[j][:],
                    xh[:, off : off + 128],
                    w_tiles[kt][:],
                    start=(kt == 0),
                    stop=(kt == KT - 1),
                )
        for j in range(H):
            store_out(j, head_psums[j])

        for r in range(H, rt):
            x_t = x_tiles[r]
            psum = p_pool.tile([128, D_OUT], f32, tag="ps", name=f"ps{r}")
            for kt in range(KT):
                nc.tensor.matmul(
                    psum[:],
                    x_t[:, bass.ts(kt, 128)],
                    w_tiles[kt][:],
                    start=(kt == 0),
                    stop=(kt == KT - 1),
                )
            store_out(r, psum)

    nc.compile()
    return nc


def make_in_maps(x, index, W, x_dt=None, w_dt=None):
    """Group rows by expert, pack per-core transposed tiles.

    Returns (in_maps, rows_per_expert, rt) where rows_per_expert[e] is the
    original row indices handled by core e.
    """
    import concourse.mybir as _mybir

    x_np = _mybir.dt.np(x_dt or X_DT)
    w_np = _mybir.dt.np(w_dt or W_DT)
    x = np.ascontiguousarray(x, dtype=np.float32)
    W = np.ascontiguousarray(W, dtype=np.float32)
    rows_per_expert = [np.nonzero(index == e)[0] for e in range(N_CORES)]
    max_rows = max(len(r) for r in rows_per_expert)
    rt = max((max_rows + 127) // 128, 1)
    r_pad = rt * 128

    in_maps = []
    for e in range(N_CORES):
        rows = rows_per_expert[e]
        xp = np.zeros((r_pad, D_IN), np.float32)
        xp[: len(rows)] = x[rows]
        # [R, D_IN] -> [RT, 128r, KT, 128k] -> [RT, 128k, KT, 128r]
        # so a partition line (fixed k) is KT*128 elements contiguous.
        xT = np.ascontiguousarray(
            xp.reshape(rt, 128, KT, 128).transpose(0, 3, 2, 1).reshape(rt, 128, -1),
            dtype=x_np,
        )
        wT = np.ascontiguousarray(W[e].T.reshape(KT, 128, D_OUT), dtype=w_np)
        in_maps.append({"xT": xT, "wT": wT})
    return in_maps, rows_per_expert, rt


def assemble_output(results, rows_per_expert, n_rows, index=None, b=None):
    y = np.zeros((n_rows, D_OUT), np.float32)
    for e, rows in enumerate(rows_per_expert):
        yc = results[e]["y"].reshape(-1, D_OUT)
        y[rows] = yc[: len(rows)]
    if b is not None and np.any(b):
        y += np.asarray(b, np.float32)[np.asarray(index)]
    return y


def kernel(x, index, W, b):
    x = np.asarray(x)
    index = np.asarray(index, np.int32)
    W = np.asarray(W)
    b = np.asarray(b)
    in_maps, rows_per_expert, rt = make_in_maps(x, index, W)
    nc = build_nc(rt)
    res = run_bass_kernel_spmd(nc, in_maps, core_ids=list(range(N_CORES)))
    return assemble_output(res.results, rows_per_expert, x.shape[0], index, b)


# revision 39
# speedup vs baseline: 1.0803x; 1.0803x over previous
"""MoE routing kernel for Trainium2 (8 NeuronCores, expert-parallel).

Problem: y[n] = x[n] @ W[index[n]].T + b[index[n]]
  x [16384, 1024] f32, index [16384] i32, W [8, 512, 1024] f32, b [8, 512] f32

Strategy (expert-parallel, dispatch on index during sharding):
  Core e owns expert e. The host groups rows by expert (the all-to-all
  dispatch), packs each core's rows into PE-friendly transposed tiles, and
  each core runs a dense [R,1024] @ [1024,512] matmul with its expert's
  weights. Results are scattered back to original row order on the host.

Device layout per core (one NEFF, SPMD on cores 0-7):
  xT  [RT, 128, 8, 128]  (row-tile, k%128, k-tile, r) — lhsT blocks; a
                         partition line (fixed k) is contiguous in DRAM
  wT  [8, 128, 512]      (k-tile, k, o)               — rhs blocks (moving)
  y   [RT, 128, 512]     (row-tile, r, o)
  For each row-tile: accumulate 8 matmuls over k-tiles into one PSUM bank,
  copy PSUM->SBUF on DVE, DMA out.
"""

from contextlib import ExitStack

import numpy as np

import concourse.bass as bass
import concourse.mybir as mybir
import concourse.tile as tile
from concourse import bacc
from concourse.bass_utils import run_bass_kernel_spmd

N_CORES = 8
D_IN = 1024
D_OUT = 512
KT = D_IN // 128  # 8 k-tiles

# matmul input dtypes (lhsT = x blocks, rhs = W blocks). float16 runs the
# PE at 1 column/cycle with fast weight load (fp32 is 4x slower, fp32r has
# no fast weight load) and halves the input DMA. Accuracy vs the fp32
# reference is ~3e-4 relative (10-bit mantissa; values here are well within
# fp16 range: |x| < ~6, |W| < ~0.06, accumulation in fp32 PSUM).
X_DT = mybir.dt.float16
W_DT = mybir.dt.float16

# Output DMA dtype. float16 halves the store traffic (HBM bandwidth is
# shared per core pair); the host upcasts back to float32. Adds at most
# 2^-11 relative rounding on top of the ~3e-4 matmul error.
Y_DT = mybir.dt.float16

# Number of PE-warmup dummy matmuls (0 disables). They run in the dead
# window between the engine-body start (~15us) and the first real matmul
# (~17.6us, gated by DMA completions), accumulating HAM busy time so the
# real matmuls run at 2.4 GHz sooner.
WARMUP_MMS = 8

# Skip the construction-time all-engine barrier (earlier first DMA).
SKIP_INIT_BARRIER = False


class _NoInitBarrierBacc(bacc.Bacc):
    """Bacc whose construction-time all-engine barrier is skipped.

    Bass.__init__ ends with an all-engine barrier whose only job is to order
    the const-pool memsets (which this kernel never reads) before the body.
    Skipping it lets each engine enter the body as soon as the runtime
    releases it, so the first DMAs issue ~4us earlier. All body dependencies
    are still fully managed by Tile's semaphores (initialized by the NEFF
    loader, not by engine code).
    """

    def all_engine_barrier(self, *, sem_only: bool = False):
        if not getattr(self, "_init_barrier_skipped", False):
            self._init_barrier_skipped = True
            return None
        return super().all_engine_barrier(sem_only=sem_only)


def build_nc(rt: int, x_dt=None, w_dt=None):
    """Build + compile the per-core Bass program for `rt` row-tiles."""
    x_dt = x_dt or X_DT
    w_dt = w_dt or W_DT
    nc = (_NoInitBarrierBacc if SKIP_INIT_BARRIER else bacc.Bacc)(
        "TRN2",
        target_bir_lowering=False,
        debug=False,
        enable_asserts=False,
        num_devices=N_CORES,
    )
    f32 = mybir.dt.float32
    xT = nc.dram_tensor("xT", [rt, 128, KT * 128], x_dt, kind="ExternalInput").ap()
    wT = nc.dram_tensor("wT", [KT, 128, D_OUT], w_dt, kind="ExternalInput").ap()
    y = nc.dram_tensor("y", [rt, 128, D_OUT], Y_DT, kind="ExternalOutput").ap()

    with tile.TileContext(nc) as tc, ExitStack() as ctx:
        w_pool = ctx.enter_context(tc.tile_pool(name="w", bufs=1))
        x_pool = ctx.enter_context(tc.tile_pool(name="x", bufs=8))
        o_pool = ctx.enter_context(tc.tile_pool(name="o", bufs=8))
        p_pool = ctx.enter_context(tc.tile_pool(name="p", bufs=6, space="PSUM"))

        w_tiles = []
        for kt in range(KT):
            w_tiles.append(
                w_pool.tile([128, D_OUT], w_dt, tag=f"w{kt}", name=f"w{kt}")
            )

        # PE warmup: the HAM clock gate keeps the PE at 1.2 GHz until it has
        # been busy ~3.4us, and re-throttles after ~3.4us idle.
        if WARMUP_MMS:
            warm_pool = ctx.enter_context(tc.tile_pool(name="warm", bufs=1))
            warm_sb = warm_pool.tile(
                [128, D_OUT], x_dt, tag="warm", name="warm_sb"
            )
            nc.vector.memset(warm_sb[:], 0.0)
            warm_ps = p_pool.tile(
                [128, D_OUT], f32, tag="warm_ps", name="warm_ps", bufs=1
            )
            for i in range(WARMUP_MMS):
                nc.tensor.matmul(
                    warm_ps[:], warm_sb[:, :128], warm_sb[:], start=True, stop=True
                )

        # Head: the first H row-tiles are processed k-major (for each
        # k-tile, H matmuls across the row-tiles). A single row-tile
        # consumes one W k-tile per 216ns, but each DMA ring completes a
        # transfer only every ~0.6-1.1us, so a row-major head stalls on W
        # arrivals and the stalls break the HAM busy window (leaving the PE
        # at 1.2 GHz for a core-dependent stretch). With H=4 matmuls per W
        # k-tile the consumption rate (~0.86us/k-tile) stays above the
        # arrival rate, so the head streams gap-free right after the warmup
        # chain. The head x block is loaded k-block-major (one DMA per
        # k-tile pair covering all H row-tiles) to match consumption order.
        H = min(4, rt)
        x_tiles = {}
        xh = x_pool.tile(
            [128, (KT // 2) * H * 256], x_dt, tag="xh", name="xh", bufs=1
        )
        for p in range(KT // 2):
            dst = xh[:, p * H * 256 : (p + 1) * H * 256]
            nc.sync.dma_start(
                dst.rearrange("k (j f) -> k j f", j=H),
                xT[0:H, :, p * 256 : (p + 1) * 256].rearrange("j k f -> k j f"),
            )
        for kt in (0, 2, 4, 6):
            nc.scalar.dma_start(w_tiles[kt][:], wT[kt])
        for kt in (1, 3, 5, 7):
            nc.gpsimd.dma_start(w_tiles[kt][:], wT[kt])
        for r in range(H, rt):
            x_t = x_pool.tile([128, KT * 128], x_dt, tag="x", name=f"x{r}")
            nc.sync.dma_start(x_t[:], xT[r])
            x_tiles[r] = x_t

        def store_out(r, psum):
            # Copy + store in halves: the second half's chain (copy ->
            # issue -> transfer -> completion) is what trails the last
            # matmul, and halving it shortens the kernel tail.
            o_t = o_pool.tile([128, D_OUT], Y_DT, tag="o", name=f"o{r}")
            half = D_OUT // 2
            for h in (0, 1):
                sl = slice(h * half, (h + 1) * half)
                nc.vector.tensor_copy(o_t[:, sl], psum[:, sl])
                nc.scalar.dma_start(y[r][:, sl], o_t[:, sl])

        head_psums = [
            p_pool.tile([128, D_OUT], f32, tag="ps", name=f"ps{j}")
            for j in range(H)
        ]
        for kt in range(KT):
            p = kt // 2
            for j in range(H):
                off = p * H * 256 + j * 256 + (kt % 2) * 128
                nc.tensor.matmul(
                    head_psums[j][:],
                    xh[:, off : off + 128],
                    w_tiles[kt][:],
                    start=(kt == 0),
                    stop=(kt == KT - 1),
                )
        for j in range(H):
            store_out(j, head_psums[j])

        for r in range(H, rt):
            x_t = x_tiles[r]
            psum = p_pool.tile([128, D_OUT], f32, tag="ps", name=f"ps{r}")
            for kt in range(KT):
                nc.tensor.matmul(
                    psum[:],
                    x_t[:, bass.ts(kt, 128)],
                    w_tiles[kt][:],
                    start=(kt == 0),
                    stop=(kt == KT - 1),
                )
            store_out(r, psum)

    nc.compile()
    return nc


def make_in_maps(x, index, W, x_dt=None, w_dt=None):
    """Group rows by expert, pack per-core transposed tiles.

    Returns (in_maps, rows_per_expert, rt) where rows_per_expert[e] is the
    original row indices handled by core e.
    """
    import concourse.mybir as _mybir

    x_np = _mybir.dt.np(x_dt or X_DT)
    w_np = _mybir.dt.np(w_dt or W_DT)
    x = np.ascontiguousarray(x, dtype=np.float32)
    W = np.ascontiguousarray(W, dtype=np.float32)
    rows_per_expert = [np.nonzero(index == e)[0] for e in range(N_CORES)]
    max_rows = max(len(r) for r in rows_per_expert)
    rt = max((max_rows + 127) // 128, 1)
    r_pad = rt * 128

    in_maps = []
    for e in range(N_CORES):
        rows = rows_per_expert[e]
        xp = np.zeros((r_pad, D_IN), np.float32)
        xp[: len(rows)] = x[rows]
        # [R, D_IN] -> [RT, 128r, KT, 128k] -> [RT, 128k, KT, 128r]
        # so a partition line (fixed k) is KT*128 elements contiguous.
        xT = np.ascontiguousarray(
            xp.reshape(rt, 128, KT, 128).transpose(0, 3, 2, 1).reshape(rt, 128, -1),
            dtype=x_np,
        )
        wT = np.ascontiguousarray(W[e].T.reshape(KT, 128, D_OUT), dtype=w_np)
        in_maps.append({"xT": xT, "wT": wT})
    return in_maps, rows_per_expert, rt


def assemble_output(results, rows_per_expert, n_rows, index=None, b=None):
    y = np.zeros((n_rows, D_OUT), np.float32)
    for e, rows in enumerate(rows_per_expert):
        yc = results[e]["y"].reshape(-1, D_OUT)
        y[rows] = yc[: len(rows)].astype(np.float32)
    if b is not None and np.any(b):
        y += np.asarray(b, np.float32)[np.asarray(index)]
    return y


def kernel(x, index, W, b):
    x = np.asarray(x)
    index = np.asarray(index, np.int32)
    W = np.asarray(W)
    b = np.asarray(b)
    in_maps, rows_per_expert, rt = make_in_maps(x, index, W)
    nc = build_nc(rt)
    res = run_bass_kernel_spmd(nc, in_maps, core_ids=list(range(N_CORES)))
    return assemble_output(res.results, rows_per_expert, x.shape[0], index, b)


# revision 41
# speedup vs baseline: 1.0897x; 1.0087x over previous
"""MoE routing kernel for Trainium2 (8 NeuronCores, expert-parallel).

Problem: y[n] = x[n] @ W[index[n]].T + b[index[n]]
  x [16384, 1024] f32, index [16384] i32, W [8, 512, 1024] f32, b [8, 512] f32

Strategy (expert-parallel, dispatch on index during sharding):
  Core e owns expert e. The host groups rows by expert (the all-to-all
  dispatch), packs each core's rows into PE-friendly transposed tiles, and
  each core runs a dense [R,1024] @ [1024,512] matmul with its expert's
  weights. Results are scattered back to original row order on the host.

Device layout per core (one NEFF, SPMD on cores 0-7):
  xT  [RT, 128, 8, 128]  (row-tile, k%128, k-tile, r) — lhsT blocks; a
                         partition line (fixed k) is contiguous in DRAM
  wT  [8, 128, 512]      (k-tile, k, o)               — rhs blocks (moving)
  y   [RT, 128, 512]     (row-tile, r, o)
  For each row-tile: accumulate 8 matmuls over k-tiles into one PSUM bank,
  copy PSUM->SBUF on DVE, DMA out.
"""

from contextlib import ExitStack

import numpy as np

import concourse.bass as bass
import concourse.mybir as mybir
import concourse.tile as tile
from concourse import bacc
from concourse.bass_utils import run_bass_kernel_spmd

N_CORES = 8
D_IN = 1024
D_OUT = 512
KT = D_IN // 128  # 8 k-tiles

# matmul input dtypes (lhsT = x blocks, rhs = W blocks). float16 runs the
# PE at 1 column/cycle with fast weight load (fp32 is 4x slower, fp32r has
# no fast weight load) and halves the input DMA. Accuracy vs the fp32
# reference is ~3e-4 relative (10-bit mantissa; values here are well within
# fp16 range: |x| < ~6, |W| < ~0.06, accumulation in fp32 PSUM).
X_DT = mybir.dt.float16
W_DT = mybir.dt.float16

# Output DMA dtype. float16 halves the store traffic (HBM bandwidth is
# shared per core pair); the host upcasts back to float32. Adds at most
# 2^-11 relative rounding on top of the ~3e-4 matmul error.
Y_DT = mybir.dt.float16

# Number of PE-warmup dummy matmuls (0 disables). They run in the dead
# window between the engine-body start (~15us) and the first real matmul
# (~17.6us, gated by DMA completions), accumulating HAM busy time so the
# real matmuls run at 2.4 GHz sooner.
WARMUP_MMS = 8

# Skip the construction-time all-engine barrier (earlier first DMA).
SKIP_INIT_BARRIER = False


class _NoInitBarrierBacc(bacc.Bacc):
    """Bacc whose construction-time all-engine barrier is skipped.

    Bass.__init__ ends with an all-engine barrier whose only job is to order
    the const-pool memsets (which this kernel never reads) before the body.
    Skipping it lets each engine enter the body as soon as the runtime
    releases it, so the first DMAs issue ~4us earlier. All body dependencies
    are still fully managed by Tile's semaphores (initialized by the NEFF
    loader, not by engine code).
    """

    def all_engine_barrier(self, *, sem_only: bool = False):
        if not getattr(self, "_init_barrier_skipped", False):
            self._init_barrier_skipped = True
            return None
        return super().all_engine_barrier(sem_only=sem_only)


def build_nc(rt: int, x_dt=None, w_dt=None):
    """Build + compile the per-core Bass program for `rt` row-tiles."""
    x_dt = x_dt or X_DT
    w_dt = w_dt or W_DT
    nc = (_NoInitBarrierBacc if SKIP_INIT_BARRIER else bacc.Bacc)(
        "TRN2",
        target_bir_lowering=False,
        debug=False,
        enable_asserts=False,
        num_devices=N_CORES,
    )
    f32 = mybir.dt.float32
    xT = nc.dram_tensor("xT", [rt, 128, KT * 128], x_dt, kind="ExternalInput").ap()
    wT = nc.dram_tensor("wT", [KT, 128, D_OUT], w_dt, kind="ExternalInput").ap()
    y = nc.dram_tensor("y", [rt, 128, D_OUT], Y_DT, kind="ExternalOutput").ap()

    with tile.TileContext(nc) as tc, ExitStack() as ctx:
        w_pool = ctx.enter_context(tc.tile_pool(name="w", bufs=1))
        x_pool = ctx.enter_context(tc.tile_pool(name="x", bufs=8))
        o_pool = ctx.enter_context(tc.tile_pool(name="o", bufs=8))
        p_pool = ctx.enter_context(tc.tile_pool(name="p", bufs=6, space="PSUM"))

        w_tiles = []
        for kt in range(KT):
            w_tiles.append(
                w_pool.tile([128, D_OUT], w_dt, tag=f"w{kt}", name=f"w{kt}")
            )

        # PE warmup: the HAM clock gate keeps the PE at 1.2 GHz until it has
        # been busy ~3.4us, and re-throttles after ~3.4us idle.
        if WARMUP_MMS:
            warm_pool = ctx.enter_context(tc.tile_pool(name="warm", bufs=1))
            warm_sb = warm_pool.tile(
                [128, D_OUT], x_dt, tag="warm", name="warm_sb"
            )
            nc.vector.memset(warm_sb[:], 0.0)
            warm_ps = p_pool.tile(
                [128, D_OUT], f32, tag="warm_ps", name="warm_ps", bufs=1
            )
            for i in range(WARMUP_MMS):
                nc.tensor.matmul(
                    warm_ps[:], warm_sb[:, :128], warm_sb[:], start=True, stop=True
                )

        # Head: the first H row-tiles are processed k-major (for each
        # k-tile, H matmuls across the row-tiles). A single row-tile
        # consumes one W k-tile per 216ns, but each DMA ring completes a
        # transfer only every ~0.6-1.1us, so a row-major head stalls on W
        # arrivals and the stalls break the HAM busy window (leaving the PE
        # at 1.2 GHz for a core-dependent stretch). With H=4 matmuls per W
        # k-tile the consumption rate (~0.86us/k-tile) stays above the
        # arrival rate, so the head streams gap-free right after the warmup
        # chain. The head x block is loaded k-block-major (one DMA per
        # k-tile pair covering all H row-tiles) to match consumption order.
        H = min(4, rt)
        x_tiles = {}
        xh = x_pool.tile(
            [128, (KT // 2) * H * 256], x_dt, tag="xh", name="xh", bufs=1
        )
        for p in range(KT // 2):
            dst = xh[:, p * H * 256 : (p + 1) * H * 256]
            nc.sync.dma_start(
                dst.rearrange("k (j f) -> k j f", j=H),
                xT[0:H, :, p * 256 : (p + 1) * 256].rearrange("j k f -> k j f"),
            )
        for kt in (0, 2, 4, 6):
            nc.scalar.dma_start(w_tiles[kt][:], wT[kt])
        for kt in (1, 3, 5, 7):
            nc.gpsimd.dma_start(w_tiles[kt][:], wT[kt])
        for r in range(H, rt):
            x_t = x_pool.tile([128, KT * 128], x_dt, tag="x", name=f"x{r}")
            nc.sync.dma_start(x_t[:], xT[r])
            x_tiles[r] = x_t

        def store_out(r, psum):
            # Copy + store in halves: the second half's chain (copy ->
            # issue -> transfer -> completion) is what trails the last
            # matmul, and halving it shortens the kernel tail.
            o_t = o_pool.tile([128, D_OUT], Y_DT, tag="o", name=f"o{r}")
            half = D_OUT // 2
            for h in (0, 1):
                sl = slice(h * half, (h + 1) * half)
                nc.vector.tensor_copy(o_t[:, sl], psum[:, sl])
                nc.scalar.dma_start(y[r][:, sl], o_t[:, sl])

        head_psums = [
            p_pool.tile([128, D_OUT], f32, tag="ps", name=f"ps{j}")
            for j in range(H)
        ]
        for kt in range(KT):
            p = kt // 2
            for j in range(H):
                off = p * H * 256 + j * 256 + (kt % 2) * 128
                nc.tensor.matmul(
                    head_psums[j][:],
                    xh[:, off : off + 128],
                    w_tiles[kt][:],
                    start=(kt == 0),
                    stop=(kt == KT - 1),
                )
        for j in range(H):
            store_out(j, head_psums[j])

        for r in range(H, rt):
            x_t = x_tiles[r]
            psum = p_pool.tile([128, D_OUT], f32, tag="ps", name=f"ps{r}")
            for kt in range(KT):
                nc.tensor.matmul(
                    psum[:],
                    x_t[:, bass.ts(kt, 128)],
                    w_tiles[kt][:],
                    start=(kt == 0),
                    stop=(kt == KT - 1),
                )
            store_out(r, psum)

    nc.compile()
    return nc


def make_in_maps(x, index, W, x_dt=None, w_dt=None):
    """Group rows by expert, pack per-core transposed tiles.

    Returns (in_maps, rows_per_expert, rt) where rows_per_expert[e] is the
    original row indices handled by core e.
    """
    import concourse.mybir as _mybir

    x_np = _mybir.dt.np(x_dt or X_DT)
    w_np = _mybir.dt.np(w_dt or W_DT)
    x = np.ascontiguousarray(x, dtype=np.float32)
    W = np.ascontiguousarray(W, dtype=np.float32)
    rows_per_expert = [np.nonzero(index == e)[0] for e in range(N_CORES)]
    max_rows = max(len(r) for r in rows_per_expert)
    rt = max((max_rows + 127) // 128, 1)
    r_pad = rt * 128

    in_maps = []
    for e in range(N_CORES):
        rows = rows_per_expert[e]
        xp = np.zeros((r_pad, D_IN), np.float32)
        xp[: len(rows)] = x[rows]
        # [R, D_IN] -> [RT, 128r, KT, 128k] -> [RT, 128k, KT, 128r]
        # so a partition line (fixed k) is KT*128 elements contiguous.
        xT = np.ascontiguousarray(
            xp.reshape(rt, 128, KT, 128).transpose(0, 3, 2, 1).reshape(rt, 128, -1),
            dtype=x_np,
        )
        wT = np.ascontiguousarray(W[e].T.reshape(KT, 128, D_OUT), dtype=w_np)
        in_maps.append({"xT": xT, "wT": wT})
    return in_maps, rows_per_expert, rt


def assemble_output(results, rows_per_expert, n_rows, index=None, b=None):
    y = np.zeros((n_rows, D_OUT), np.float32)
    for e, rows in enumerate(rows_per_expert):
        yc = results[e]["y"].reshape(-1, D_OUT)
        y[rows] = yc[: len(rows)].astype(np.float32)
    if b is not None and np.any(b):
        y += np.asarray(b, np.float32)[np.asarray(index)]
    return y


def kernel(x, index, W, b):
    x = np.asarray(x)
    index = np.asarray(index, np.int32)
    W = np.asarray(W)
    b = np.asarray(b)
    in_maps, rows_per_expert, rt = make_in_maps(x, index, W)
    nc = build_nc(rt)
    res = run_bass_kernel_spmd(nc, in_maps, core_ids=list(range(N_CORES)))
    return assemble_output(res.results, rows_per_expert, x.shape[0], index, b)
